# revision 29
# baseline (speedup 1.0000x reference)
"""MoE MLP (top-1 routing) on 8 TRN2 NeuronCores.

Strategy (expert-parallel, per the sharding hint): the host computes the
router argmax and dispatches each token to its expert's core. Core e holds
w_fc[e]/w_proj[e] and runs the dense expert MLP
    y = lrelu_0.5(x_fc)^2 @ w_proj[e].T,   x_fc = x @ w_fc[e].T
on its gathered tokens in a transposed (feature-major) layout so no on-device
transposes are needed.

All matmul operands are bf16 (PE streams 1 column/cycle — same rate as
float32r — while halving HBM traffic and SBUF footprint); PSUM accumulation
stays fp32 and y is stored fp32. lrelu^2 is computed as (max(0.5*p, p))^2:
one DVE scalar_tensor_tensor + one ACT Square per group.

Tokens are processed in near-equal blocks of <=512. x/y DRAM layouts are
block-major so every transfer is contiguous per partition. x loads issue on
the SP HWDGE queue; weights and y stores on the ACT HWDGE queue, so the
next iteration's x prefetch is never stuck behind output stores. y is
written j-major per block and stored in quarter-block chunks as the GEMM2
j-groups complete, so the final store exposes only ~1us. The emission order
software-pipelines blocks on the PE: GEMM2 of block b-1 is issued between
GEMM1 m-groups of block b, so the PE never waits on the ACT/DVE chain.
"""

import contextlib
import numpy as np
from ml_dtypes import bfloat16

import concourse.mybir as mybir
import concourse.tile as tile
from concourse import bacc
from concourse.bass_utils import run_bass_kernel_spmd

P = 128          # SBUF partitions / PE array dim
D = 1024         # model dim
E = 8            # experts == cores
H = 512          # expert hidden dim
KD = D // P      # k-chunks over D
KH = H // P      # k-chunks over H
MD = D // P      # output d-tiles
TB = 512         # max token block (PSUM bank limit for fp32 outputs)

F32 = mybir.dt.float32
BF16 = mybir.dt.bfloat16
AF = mybir.ActivationFunctionType
ALU = mybir.AluOpType

_programs = {}
last_exec_ns = None
_ALT_COPY = True   # GEMM2 evacuation alternates DVE/ACT
_PSUM_HP = 2   # GEMM1 PSUM banks
_PSUM_YP = 5   # GEMM2 PSUM banks (GEMM2's j-group bursts need the slack;
               # GEMM1's banks drain fast through the ACT/DVE chain)


def _token_blocks(C):
    # Near-equal blocks of at most TB tokens (multiples of 128).
    chunks = C // P
    nb = -(-chunks // (TB // P))
    q, r = divmod(chunks, nb)
    sizes = [(q + 1) * P] * r + [q * P] * (nb - r)
    blocks = []
    t = 0
    for tb in sizes:
        blocks.append((t, tb))
        t += tb
    return blocks


def _build_program(C, repeat=1, unroll=1, mode="full", ydt=F32):
    nc = bacc.Bacc("TRN2", target_bir_lowering=False, debug=False)
    xk = nc.declare_dram_parameter("xk", [P, KD * C], BF16, isOutput=False)
    wfck = nc.declare_dram_parameter("wfck", [P, KH, KD * P], BF16,
                                     isOutput=False)
    wpjk = nc.declare_dram_parameter("wpjk", [P, KH, D], BF16, isOutput=False)
    yk = nc.declare_dram_parameter("yk", [P, C * MD], ydt, isOutput=True)

    blocks = _token_blocks(C)

    with tile.TileContext(nc) as tc:
        with (
            tc.tile_pool(name="wpool", bufs=1) as wpool,
            tc.tile_pool(name="xpool", bufs=3) as xpool,
            tc.tile_pool(name="hpool", bufs=3) as hpool,
            tc.tile_pool(name="ypool", bufs=4) as ypool,
            tc.tile_pool(name="spool", bufs=3) as spool,
            tc.tile_pool(name="hpsum", bufs=_PSUM_HP, space="PSUM") as hpsum,
            tc.tile_pool(name="ypsum", bufs=_PSUM_YP, space="PSUM") as ypsum,
            contextlib.ExitStack() as loop_ctx,
        ):
            # Weights load on the ACT queue, per-m-group chunks for wfc so
            # the first GEMM1 group can start after a 256KB transfer.
            wfc_sb = wpool.tile([P, KH, KD * P], BF16)
            for m in range(KH):
                nc.scalar.dma_start(wfc_sb[:, m, :], wfck[:, m, :])
            wpj_sb = wpool.tile([P, KH, D], BF16)
            nc.scalar.dma_start(wpj_sb[:], wpjk[:])

            if repeat > 1:
                loop_ctx.enter_context(
                    tc.For_i(0, repeat, 1,
                             hint_engines=(mybir.EngineType.PE,)))

            def g1_group(x_sb, h_sb, tb, m):
                ph = hpsum.tile([P, tb], F32, tag="ph")
                for k in range(KD):
                    nc.tensor.matmul(
                        ph[:],
                        wfc_sb[:, m, k * P:(k + 1) * P],
                        x_sb[:, k, :],
                        start=(k == 0),
                        stop=(k == KD - 1),
                    )
                if mode == "g1only":
                    return
                # h = lrelu_0.5(ph)^2 = (0.5*(relu(ph) + ph))^2
                # (a DVE op may read only ONE input from PSUM, and ACT
                # Lrelu's alpha operand is broken on HW, hence 3 ops.)
                r_sb = spool.tile([P, tb], F32, tag="r")
                nc.scalar.activation(r_sb[:], ph[:], AF.Relu)
                s_sb = spool.tile([P, tb], F32, tag="s")
                nc.vector.scalar_tensor_tensor(
                    s_sb[:], r_sb[:], 0.0, ph[:], ALU.add, ALU.add)
                nc.scalar.activation(
                    h_sb[:, m, :], s_sb[:], AF.Square, scale=0.5)

            def g2_group(h_sb, y_blk, tb, j):
                py = ypsum.tile([P, tb], F32, tag="py")
                for kh in range(KH):
                    nc.tensor.matmul(
                        py[:],
                        wpj_sb[:, kh, j * P:(j + 1) * P],
                        h_sb[:, kh, :],
                        start=(kh == 0),
                        stop=(kh == KH - 1),
                    )
                # evacuate on alternating engines (both can read PSUM) so
                # the j-group bursts free their banks twice as fast
                if _ALT_COPY and j % 2 == 1:
                    nc.scalar.activation(y_blk[:, j, :], py[:], AF.Copy)
                else:
                    nc.vector.tensor_copy(y_blk[:, j, :], py[:])

            def y_store_pair(t0, tb, y_blk, j):
                # store j-pair [j-1, j] of the j-major block region
                dst = yk[:, MD * t0 + (j - 1) * tb: MD * t0 + (j + 1) * tb]
                nc.scalar.dma_start(
                    dst.rearrange("p (j t) -> p j t", j=2),
                    y_blk[:, j - 1:j + 1, :])

            # Software pipeline with fine interleave: between the m-groups
            # of GEMM1(b), emit the j-groups of GEMM2(b-1), so the PE always
            # has independent work while block b's epilogue runs.
            def emit_blocks(prev):
                for bi, (t0, tb) in enumerate(blocks):
                    x_sb = xpool.tile([P, KD, tb], BF16, tag="x")
                    src = xk[:, KD * t0:KD * (t0 + tb)].rearrange(
                        "p (k t) -> p k t", k=KD)
                    if bi == 0 and prev is None:
                        # head-latency trim: split the first block's load
                        # across both HWDGE queues (they run in parallel)
                        kh2 = KD // 2
                        nc.sync.dma_start(x_sb[:, :kh2, :], src[:, :kh2, :])
                        nc.scalar.dma_start(x_sb[:, kh2:, :], src[:, kh2:, :])
                    else:
                        nc.sync.dma_start(x_sb[:], src)
                    h_sb = hpool.tile([P, KH, tb], BF16, tag="h")
                    if prev is not None:
                        p0, ptb, ph_sb = prev
                        y_blk = ypool.tile([P, MD, ptb], ydt, tag="y")
                    # Defer prev's GEMM2 by one m-group: h(prev) is finished
                    # by the ACT/DVE chain ~2.7us after prev's last GEMM1
                    # drains, so give it two g1 groups (~3.4us) of PE work
                    # plus the b-2 leftovers before the first g2 needs it.
                    for m in range(KH):
                        g1_group(x_sb, h_sb, tb, m)
                        if prev is not None and mode == "full":
                            if m == 1:
                                for j in (0, 2):
                                    g2_group(ph_sb, y_blk, ptb, j)
                                    g2_group(ph_sb, y_blk, ptb, j + 1)
                                    y_store_pair(p0, ptb, y_blk, j + 1)
                            elif m >= 2:
                                g2_group(ph_sb, y_blk, ptb, 2 * m)
                                g2_group(ph_sb, y_blk, ptb, 2 * m + 1)
                                y_store_pair(p0, ptb, y_blk, 2 * m + 1)
                    prev = (t0, tb, h_sb)
                return prev

            def flush(prev):
                p0, ptb, ph_sb = prev
                y_blk = ypool.tile([P, MD, ptb], ydt, tag="y")
                for j in range(MD):
                    if j >= MD // 2:
                        # tail-latency trim: last j-groups evacuate on
                        # alternating engines (ACT can also read PSUM) and
                        # stream out singly on alternating HWDGE queues so
                        # the final copies and drains overlap.
                        py = ypsum.tile([P, ptb], F32, tag="py")
                        for kh in range(KH):
                            nc.tensor.matmul(
                                py[:],
                                wpj_sb[:, kh, j * P:(j + 1) * P],
                                ph_sb[:, kh, :],
                                start=(kh == 0),
                                stop=(kh == KH - 1),
                            )
                        if j % 2 == 0:
                            nc.vector.tensor_copy(y_blk[:, j, :], py[:])
                        else:
                            nc.scalar.activation(
                                y_blk[:, j, :], py[:], AF.Copy)
                        dst = yk[:, MD * p0 + j * ptb:
                                 MD * p0 + (j + 1) * ptb]
                        eng = nc.sync if j % 2 == 0 else nc.scalar
                        eng.dma_start(dst, y_blk[:, j, :])
                    else:
                        g2_group(ph_sb, y_blk, ptb, j)
                        if j % 2 == 1:
                            y_store_pair(p0, ptb, y_blk, j)

            prev = None
            for _ in range(unroll):
                prev = emit_blocks(prev)
            if mode == "full":
                flush(prev)

    nc.compile()
    return nc


def _program(C):
    if C not in _programs:
        _programs[C] = _build_program(C)
    return _programs[C]


def _pack_x(xg, C):
    """[C, D] bf16 tokens -> block-major [P, KD*C] bf16 array."""
    parts = []
    for (t0, tb) in _token_blocks(C):
        blk = xg[t0:t0 + tb].reshape(tb, KD, P).transpose(2, 1, 0)
        parts.append(blk.reshape(P, KD * tb))
    return np.ascontiguousarray(np.concatenate(parts, axis=1))


def _pack_wfc(wfc_e):
    """[H, D] -> [P, KH, KD*P] bf16, m-group-major."""
    w = wfc_e.reshape(KH, P, KD, P)          # [m, hh, k, p]
    return np.ascontiguousarray(
        w.transpose(3, 0, 2, 1).reshape(P, KH, KD * P).astype(bfloat16))


def _pack_wproj(wproj_e):
    return np.ascontiguousarray(
        wproj_e.T.reshape(KH, P, D).transpose(1, 0, 2).astype(bfloat16))


def _unpack_y(yk_arr, C):
    """[P, C*MD] j-major blocks -> [C, D]."""
    out = np.empty((C, D), np.float32)
    for (t0, tb) in _token_blocks(C):
        seg = yk_arr[:, MD * t0:MD * (t0 + tb)].reshape(P, MD, tb)
        out[t0:t0 + tb] = seg.transpose(2, 1, 0).reshape(tb, D)
    return out


def _make_in_maps(xf16, order, offs, counts, w_fc, w_proj, C):
    in_maps = []
    for e in range(E):
        ids = order[offs[e]:offs[e + 1]]
        xg = np.zeros((C, D), bfloat16)
        xg[:len(ids)] = xf16[ids]
        in_maps.append({
            "xk": _pack_x(xg, C),
            "wfck": _pack_wfc(w_fc[e]),
            "wpjk": _pack_wproj(w_proj[e]),
        })
    return in_maps


def kernel(x, w_router, w_fc, w_proj):
    global last_exec_ns
    x = np.asarray(x, dtype=np.float32)
    w_router = np.asarray(w_router, dtype=np.float32)
    w_fc = np.asarray(w_fc, dtype=np.float32)
    w_proj = np.asarray(w_proj, dtype=np.float32)

    B, S, _ = x.shape
    N = B * S
    xf = np.ascontiguousarray(x.reshape(N, D))

    # Host-side router: top-1 expert per token (softmax is monotone, so
    # argmax over logits == argmax over softmax weights).
    logits = xf @ w_router.T
    eidx = np.argmax(logits, axis=1)
    counts = np.bincount(eidx, minlength=E)
    order = np.argsort(eidx, kind="stable")
    offs = np.concatenate(([0], np.cumsum(counts)))

    C = max(P, -(-int(counts.max()) // P) * P)  # round up to 128

    xf16 = xf.astype(bfloat16)
    in_maps = _make_in_maps(xf16, order, offs, counts, w_fc, w_proj, C)

    nc = _program(C)
    res = run_bass_kernel_spmd(nc, in_maps, core_ids=list(range(E)))
    last_exec_ns = res.exec_time_ns

    out = np.zeros((N, D), np.float32)
    for e in range(E):
        ids = order[offs[e]:offs[e + 1]]
        yg = _unpack_y(np.asarray(res.results[e]["yk"]), C)
        out[ids] = yg[:counts[e]]
    return out.reshape(B, S, D)
